# revision 36
# baseline (speedup 1.0000x reference)
"""Trainium2 Bass kernel for a pre-norm transformer block (RMSNorm + causal
RoPE attention + RMSNorm + SiLU FFN), distributed over 8 NeuronCores.

Sharding: phase 1 is head-parallel (2 of 16 heads per core, all tokens);
phase 2 is a single ~1MB-per-core AllToAll that redistributes attention
outputs from head-shards to token-shards; phase 3 (out-proj, residuals,
FFN) is token-parallel (512 of 4096 tokens per core).

All activations are kept feature-major ([feature partitions x token free])
so every matmul contraction runs over the partition axis with no on-chip
transposes of activations; x is fed pre-transposed (and pre-normalized for
the attention branch: rmsnorm-1's per-token rstd depends only on the input
x, so it is folded on the host) and the final output is transposed back on
the host.

RoPE is applied as rot(q) = q*cos_tab + shuffle(q)*sin_tab with head dims
host-permuted to [evens; odds] (leaves attention scores invariant), so it
is plain elementwise DVE work on contiguous partitions.

Softmax normalization is deferred through the AllToAll: each 130-row chunk
carries 2x(64 numerator rows + 1 denominator row); the reciprocal+scale
happens once on the token-sharded side (one DVE reciprocal for all 16
heads instead of 16 per-step ones).
"""

import sys
import time
import numpy as np
import ml_dtypes
from contextlib import ExitStack

for _p in ("/opt/trn_rl_repo", "/root/.axon_site/_ro/trn_rl_repo"):
    if _p not in sys.path:
        sys.path.insert(0, _p)

import concourse.bass as bass
import concourse.tile as tile
from concourse import mybir

F32 = mybir.dt.float32
BF16 = mybir.dt.bfloat16
FP8 = mybir.dt.float8e4
BF = ml_dtypes.bfloat16
F8 = ml_dtypes.float8_e4m3
DR = mybir.MatmulPerfMode.DoubleRow
W8SCALE = 32.0   # fp8 weights are pre-scaled by this on the host
AF = mybir.ActivationFunctionType
MUL = mybir.AluOpType.mult

B, S, D, H, DH = 2, 2048, 1024, 16, 64
FF = 2048
NCORES = 8
TLOC = (B * S) // NCORES          # 512 tokens per core in phase 3
NQB = S // 512                    # 4 query blocks of 512 per batch
NDC = D // 128                    # 8 feature chunks
NFC = FF // 128                   # 16 ffn chunks
CROWS = 130                       # A2A chunk rows: 2*(64 num + 1 den)
SCALE = 1.0 / float(np.sqrt(DH))
EPS = 1e-6
ROPE_BASE = 10000.0

_MAX_WAITS = 1


def _split_excess_waits(nc, max_waits=_MAX_WAITS):
    """walrus rejects >~2 sync-wait commands on one instruction; split the
    extras onto NoOps inserted just before, on the same engine."""
    counter = [0]

    def fresh_nop(engine, waits):
        counter[0] += 1
        nop = mybir.InstNoOp(name=f"I-waitsplit-{counter[0]}")
        nop.engine = engine
        nop.sync_info = mybir.SyncInfo(on_wait=list(waits), on_update=[])
        return nop

    for f in nc.m.functions:
        for bb in f.blocks:
            new_insts = []
            changed = False
            for inst in bb.instructions:
                si = inst.sync_info
                if si is not None and si.on_wait and len(si.on_wait) > max_waits:
                    waits = list(si.on_wait)
                    while len(waits) > max_waits:
                        chunk, waits = waits[:max_waits], waits[max_waits:]
                        new_insts.append(fresh_nop(inst.engine, chunk))
                    inst.sync_info = mybir.SyncInfo(
                        on_wait=waits, on_update=list(si.on_update or [])
                    )
                    changed = True
                new_insts.append(inst)
            if changed:
                bb.instructions[:] = new_insts
    return nc


def _build_nc(debug=False):
    nc = bass.Bass("TRN2", target_bir_lowering=False, debug=False, num_devices=NCORES)

    xt_bf = nc.dram_tensor("xt_bf", [B, D, S], FP8, kind="ExternalInput")
    xt_loc = nc.dram_tensor("xt_loc", [D, TLOC], F32, kind="ExternalInput")
    wqkv = nc.dram_tensor("wqkv", [128, 3, NDC, 128], FP8, kind="ExternalInput")
    wout = nc.dram_tensor("wout", [128, NDC, NDC, 128], FP8, kind="ExternalInput")
    wfc = nc.dram_tensor("wfc", [128, NDC, NFC, 128], BF16, kind="ExternalInput")
    wproj = nc.dram_tensor("wproj", [128, NFC, NDC, 128], BF16, kind="ExternalInput")
    cs_t = nc.dram_tensor("cs_t", [128, S], BF16, kind="ExternalInput")
    sn_t = nc.dram_tensor("sn_t", [128, S], BF16, kind="ExternalInput")
    sel_t = nc.dram_tensor("sel_t", [16, NDC * 128], F32, kind="ExternalInput")
    ident_in = nc.dram_tensor("ident_in", [128, 128], BF16, kind="ExternalInput")

    out_loc = nc.dram_tensor("out_loc", [D, TLOC], F32, kind="ExternalOutput")
    if debug:
        dbg_qrot = nc.dram_tensor("dbg_qrot", [B, 128, S], BF16, kind="ExternalOutput")
        dbg_krot = nc.dram_tensor("dbg_krot", [B, 128, S], BF16, kind="ExternalOutput")
        dbg_att = nc.dram_tensor("dbg_att", [D, TLOC], BF16, kind="ExternalOutput")
        dbg_x1 = nc.dram_tensor("dbg_x1", [D, TLOC], F32, kind="ExternalOutput")

    with tile.TileContext(nc) as tc, ExitStack() as top:
        dram = top.enter_context(tc.tile_pool(name="dram", bufs=1, space="DRAM"))

        # token-split A2A: two halves so the second collective overlaps the
        # first half's phase-3 compute
        TH = TLOC // 2
        cc_in = dram.tile([2, NCORES * CROWS, TH], BF16, tag="cc_in", name="cc_in")
        cc_out = dram.tile([2, NCORES * CROWS, TH], BF16, tag="cc_out", name="cc_out")

        consts = top.enter_context(tc.tile_pool(name="consts", bufs=1))
        ones_row = consts.tile([1, 128], F32, tag="ones_row")
        nc.vector.memset(ones_row[:], 1.0)
        ones_col = consts.tile([128, 1], BF16, tag="ones_col")
        nc.vector.memset(ones_col[:], 1.0)
        eps_sb = consts.tile([1, 1], F32, tag="eps_sb")
        nc.vector.memset(eps_sb[:], EPS)
        ident = consts.tile([128, 128], BF16, tag="ident")
        nc.sync.dma_start(ident[:], ident_in[:])
        sel_sb = consts.tile([16, NDC * 128], F32, tag="sel")
        c_w8 = consts.tile([128, 1], F32, tag="c_w8")
        nc.vector.memset(c_w8[:], 1.0 / W8SCALE)

        wpre = top.enter_context(tc.tile_pool(name="wpre", bufs=1))
        wout_sb = wpre.tile([128, NDC, NDC, 128], FP8, tag="wout")
        wfc_sb = wpre.tile([128, NDC, NFC, 128], BF16, tag="wfc")
        wproj_sb = wpre.tile([128, NFC, NDC, 128], BF16, tag="wproj")

        # ================= phase 1: head-parallel attention =================
        with ExitStack() as p1:
            # psum budget: sc (2 banks x 2 bufs) + qkv/tr ring (2) + num (2) = 8
            ps_sc = p1.enter_context(tc.tile_pool(name="ps_sc", bufs=2, space="PSUM"))
            ps_mm = p1.enter_context(tc.tile_pool(name="ps_mm", bufs=2, space="PSUM"))
            ps_num = p1.enter_context(tc.tile_pool(name="ps_num", bufs=1, space="PSUM"))
            persist = p1.enter_context(tc.tile_pool(name="persist", bufs=1))
            cs_sb = persist.tile([128, S], BF16, tag="cs")
            sn_sb = persist.tile([128, S], BF16, tag="sn")
            wqkv_sb = persist.tile([128, 3, NDC, 128], FP8, tag="wqkv")
            nc.sync.dma_start(wqkv_sb[:], wqkv[:])

            bp = p1.enter_context(tc.tile_pool(name="bp", bufs=1))
            xq = p1.enter_context(tc.tile_pool(name="xq", bufs=4))
            tmp2 = p1.enter_context(tc.tile_pool(name="tmp2", bufs=3))
            att_sb_pool = p1.enter_context(tc.tile_pool(name="attsb", bufs=4))
            ptp = p1.enter_context(tc.tile_pool(name="ptp", bufs=4))

            steps = [(b, qb) for b in range(B) for qb in range(NQB)]
            batch_tiles = {}
            x_tiles = {}

            def ensure_x(si):
                if si >= len(steps) or steps[si] in x_tiles:
                    return
                b, qb = steps[si]
                qs = slice(qb * 512, (qb + 1) * 512)
                x_q = xq.tile([128, NDC, 512], FP8, tag="x_q", name="x_q")
                src = xt_bf[b, :, qs].rearrange("(c p) f -> p c f", p=128)
                if si < 2:
                    # startup-critical: per-pair chunks so the first DoubleRow
                    # matmul starts after a quarter of the load
                    for e in range(NDC // 2):
                        nc.sync.dma_start(
                            x_q[:, 2 * e:2 * e + 2, :], src[:, 2 * e:2 * e + 2, :]
                        )
                else:
                    nc.sync.dma_start(x_q[:], src)
                x_tiles[steps[si]] = x_q

            def ensure_batch_tiles(b):
                if b not in batch_tiles:
                    q_rot = bp.tile([128, S], BF16, tag="q_rot", name="q_rot", bufs=2)
                    k_rot = bp.tile([128, S], BF16, tag="k_rot", name="k_rot", bufs=2)
                    v_aug = bp.tile([128, S // 128, 2, 65], BF16, tag="v_aug", name="v_aug", bufs=2)
                    nc.vector.memset(v_aug[:, :, :, 64:65], 1.0)
                    batch_tiles[b] = (q_rot, k_rot, v_aug)
                return batch_tiles[b]

            SWAP16 = list(range(16, 32)) + list(range(16))

            def qkv_block(b, qb):
                """qkv projections (x pre-normalized on host) + rope + v transpose."""
                qs = slice(qb * 512, (qb + 1) * 512)
                q_rot, k_rot, v_aug = ensure_batch_tiles(b)
                x_q = x_tiles.pop((b, qb))

                def project(rc):
                    # fp8 DoubleRow: each MM contracts a pair of 128-row chunks
                    mm_ps = ps_mm.tile([128, 512], F32, tag="mm", name="mm_ps")
                    for e in range(NDC // 2):
                        nc.tensor.matmul(
                            mm_ps[:], wqkv_sb[:, rc, 2 * e:2 * e + 2, :],
                            x_q[:, 2 * e:2 * e + 2, :],
                            start=(e == 0), stop=(e == NDC // 2 - 1),
                            perf_mode=DR,
                        )
                    return mm_ps

                for rc_base, dst in ((0, q_rot), (1, k_rot)):
                    p_main = project(rc_base)
                    t1 = tmp2.tile([128, 512], BF16, tag="t1", name="t1")
                    nc.vector.tensor_mul(t1[:], p_main[:], cs_sb[:, qs])
                    qsh = tmp2.tile([128, 512], F32, tag="qsh", name="qsh")
                    nc.vector.stream_shuffle(qsh[:], p_main[:], SWAP16)
                    t2 = tmp2.tile([128, 512], BF16, tag="t2", name="t2")
                    nc.vector.tensor_mul(t2[:], qsh[:], sn_sb[:, qs])
                    nc.vector.tensor_add(dst[:, qs], t1[:], t2[:])

                p_v = project(2)
                v_f = tmp2.tile([128, 512], BF16, tag="t1", name="v_f")
                nc.vector.tensor_scalar_mul(v_f[:], p_v[:], 1.0 / W8SCALE)
                tr_ps = ps_mm.tile([128, 4, 128], BF16, tag="mm", name="tr_ps")
                for j in range(4):
                    # start=True zeroes the whole 2KB psum region, so only the
                    # first transpose of the group may set it
                    nc.tensor.matmul(
                        tr_ps[:, j, :], v_f[:, j * 128:(j + 1) * 128], ident[:],
                        is_transpose=True, start=(j == 0), stop=(j == 3),
                    )
                nc.vector.tensor_copy(
                    v_aug[:, qb * 4:(qb + 1) * 4, :, 0:64],
                    tr_ps[:].rearrange("p a (h d) -> p a h d", h=2),
                )

            # software pipeline: x DMA 2+ steps ahead, qkv 1 step ahead
            ensure_x(0)
            ensure_x(1)
            nc.sync.dma_start(cs_sb[:], cs_t[:])
            nc.sync.dma_start(sn_sb[:], sn_t[:])
            nc.sync.dma_start(sel_sb[:], sel_t[:])
            ensure_x(2)
            qkv_block(*steps[0])

            for si, (b, qb) in enumerate(steps):
                qs = slice(qb * 512, (qb + 1) * 512)
                q_rot, k_rot, v_aug = batch_tiles[b]

                ensure_x(si + 3)
                if si + 1 < len(steps):
                    qkv_block(*steps[si + 1])
                if si == 1:
                    # big phase-3 weights: issue behind step-0's gpsimd work so
                    # their transfers don't steal DMA bandwidth from the
                    # startup-critical loads
                    nc.gpsimd.dma_start(wout_sb[:], wout[:])
                    nc.gpsimd.dma_start(wfc_sb[:], wfc[:])
                    nc.gpsimd.dma_start(wproj_sb[:], wproj[:])

                # ---- causal attention for this query block ----
                nkc = 4 * (qb + 1)
                num_h = [
                    ps_num.tile([65, 512], F32, tag=f"num{hh}", name=f"num{hh}")
                    for hh in range(2)
                ]
                # diagonal chunks first: the epilogue then waits on a chain
                # without the gpsimd mask step; first in order (kc=4*qb,
                # dlt=0) is full-width so start=True clears the whole bank
                kc_order = list(range(qb * 4, nkc)) + list(range(0, qb * 4))
                for ki, kc in enumerate(kc_order):
                    ks = slice(kc * 128, (kc + 1) * 128)
                    dlt = kc - qb * 4
                    qlo = max(dlt, 0) * 128     # fully-masked query columns
                    sc_ps = ps_sc.tile([128, 2, 512], F32, tag="sc", name="sc_ps")
                    for hh in range(2):
                        hs = slice(hh * 64, (hh + 1) * 64)
                        nc.tensor.matmul(
                            sc_ps[:, hh, qlo:], k_rot[hs, ks],
                            q_rot[hs, qb * 512 + qlo:(qb + 1) * 512],
                            start=True, stop=True,
                            tile_position=(hh * 64, 0),
                        )
                    pT = ptp.tile([128, 2, 512], BF16, tag="pT", name="pT")
                    nc.scalar.activation(
                        out=pT[:, :, qlo:], in_=sc_ps[:, :, qlo:],
                        func=AF.Exp, scale=SCALE,
                    )
                    if dlt >= 0:
                        # causal: zero probs where query < key (only the first
                        # 128 surviving columns can be masked)
                        nc.gpsimd.affine_select(
                            out=pT[:, :, qlo:qlo + 128], in_=pT[:, :, qlo:qlo + 128],
                            compare_op=mybir.AluOpType.is_ge,
                            fill=0.0, base=0,
                            channel_multiplier=-1,
                            pattern=[[0, 2], [1, 128]],
                        )
                    for hh in range(2):
                        nc.tensor.matmul(
                            num_h[hh][:, qlo:], v_aug[:, kc, hh, :], pT[:, hh, qlo:],
                            start=(ki == 0), stop=(ki == nkc - 1),
                        )
                # raw numerators + denominator row go straight to the A2A
                shard = b * NQB + qb
                for hh in range(2):
                    att_sb = att_sb_pool.tile([65, 512], BF16, tag="att_sb", name="att_sb")
                    nc.vector.tensor_copy(att_sb[:], num_h[hh][:])
                    rows = slice(shard * CROWS + hh * 65, shard * CROWS + (hh + 1) * 65)
                    for th in range(2):
                        nc.sync.dma_start(
                            cc_in[th, rows, :],
                            att_sb[:, th * TH:(th + 1) * TH],
                        )

                if debug and qb == NQB - 1:
                    nc.sync.dma_start(dbg_qrot[b], q_rot[:])
                    nc.sync.dma_start(dbg_krot[b], k_rot[:])

        # ================= phase 2: AllToAll (token-split x2) ==============
        for th in range(2):
            nc.gpsimd.collective_compute(
                "AllToAll",
                mybir.AluOpType.bypass,
                replica_groups=[list(range(NCORES))],
                ins=[cc_in[th]],
                outs=[cc_out[th]],
            )

        # ================= phase 3: token-parallel out-proj + FFN ==========
        # processed in two token-halves: half 0's compute overlaps half 1's
        # collective
        with ExitStack() as p3:
            wp = p3.enter_context(tc.tile_pool(name="wp", bufs=1))
            tmp3 = p3.enter_context(tc.tile_pool(name="tmp3", bufs=4))
            ps_out = p3.enter_context(tc.tile_pool(name="ps_out", bufs=4, space="PSUM"))
            ps_sel = p3.enter_context(tc.tile_pool(name="ps_sel", bufs=2, space="PSUM"))
            ps_var = p3.enter_context(tc.tile_pool(name="ps_var", bufs=2, space="PSUM"))

            xl = wp.tile([128, NDC, TLOC], F32, tag="xl")
            for dc in range(NDC):
                nc.sync.dma_start(
                    xl[:, dc, :], xt_loc[dc * 128:(dc + 1) * 128, :]
                )

            def phase3_half(th):
                """full token-half pipeline (out-proj + FFN); half 0 overlaps
                the second A2A."""
                ts = slice(th * TH, (th + 1) * TH)
                # denominators: rows 64+65*j of this half's cc_out
                den_bf = wp.tile([16, TH], BF16, tag=f"den{th}")
                nc.sync.dma_start(
                    den_bf[:],
                    cc_out[th].rearrange("(j p) f -> j p f", p=65)[:, 64, :],
                )
                rcp16 = wp.tile([16, TH], F32, tag=f"rcp16{th}")
                nc.vector.reciprocal(rcp16[:], den_bf[:])

                at_raw = wp.tile([128, NDC, TH], BF16, tag=f"at_raw{th}")
                for dc in range(NDC):
                    for hh in range(2):
                        nc.sync.dma_start(
                            at_raw[64 * hh:64 * (hh + 1), dc, :],
                            cc_out[th, dc * CROWS + hh * 65: dc * CROWS + hh * 65 + 64, :],
                        )
                at_n = wp.tile([128, NDC, TH], FP8, tag=f"at_n{th}")
                x1_all = wp.tile([128, NDC, TH], F32, tag=f"x1{th}")
                h2_all = wp.tile([128, NDC, TH], BF16, tag=f"h2{th}")
                hid_all = wp.tile([128, NFC, TH], BF16, tag=f"hid{th}")

                var2 = ps_var.tile([1, TH], F32, tag="var2", name=f"var2_{th}")
                op_ps = {}
                for g in range(2):
                    ecs = range(g * 4, g * 4 + 4)
                    for ec in ecs:
                        op_ps[ec] = ps_out.tile([128, TH], F32, tag="op", name=f"op{ec}_{th}")
                    for dc in range(NDC):
                        if g == 0:
                            sel_ps = ps_sel.tile([128, TH], F32, tag="sel", name=f"sel_{th}")
                            nc.tensor.matmul(
                                sel_ps[:], sel_sb[:, dc * 128:(dc + 1) * 128], rcp16[:],
                                start=True, stop=True,
                            )
                            nc.vector.tensor_mul(at_n[:, dc, :], at_raw[:, dc, :], sel_ps[:])
                        if dc % 2 == 1:
                            for ec in ecs:
                                nc.tensor.matmul(
                                    op_ps[ec][:], wout_sb[:, dc - 1:dc + 1, ec, :],
                                    at_n[:, dc - 1:dc + 1, :],
                                    start=(dc == 1), stop=(dc == NDC - 1),
                                    perf_mode=DR,
                                )
                    for ec in ecs:
                        nc.vector.scalar_tensor_tensor(
                            out=x1_all[:, ec, :], in0=op_ps[ec][:], scalar=c_w8[:],
                            in1=xl[:, ec, ts], op0=MUL, op1=mybir.AluOpType.add,
                        )
                        sq2 = tmp3.tile([128, TH], BF16, tag="sq2", name=f"sq2_{th}")
                        nc.vector.tensor_mul(sq2[:], x1_all[:, ec, :], x1_all[:, ec, :])
                        nc.tensor.matmul(
                            var2[:], ones_col[:], sq2[:],
                            start=(ec == 0), stop=(ec == NDC - 1),
                        )

                # rmsnorm 2 (w_ff_norm folded into wfc on host)
                sd2 = tmp3.tile([1, TH], F32, tag="sd2")
                nc.scalar.activation(
                    out=sd2[:], in_=var2[:], func=AF.Sqrt, scale=1.0 / D, bias=eps_sb[:]
                )
                rstd2 = tmp3.tile([1, TH], F32, tag="rstd2")
                nc.vector.reciprocal(rstd2[:], sd2[:])
                rstd2_ps = ps_sel.tile([128, TH], F32, tag="sel", name=f"rstd2_ps{th}")
                nc.tensor.matmul(rstd2_ps[:], ones_row[:], rstd2[:], start=True, stop=True)
                rstd2_bc = tmp3.tile([128, TH], F32, tag="rstd2_bc")
                nc.vector.tensor_copy(rstd2_bc[:], rstd2_ps[:])
                for ec in range(NDC):
                    nc.vector.tensor_mul(h2_all[:, ec, :], x1_all[:, ec, :], rstd2_bc[:])

                # fc + silu
                for fi in range(NFC):
                    fc_ps = ps_out.tile([128, TH], F32, tag="op", name=f"fc{fi}_{th}")
                    for ec in range(NDC):
                        nc.tensor.matmul(
                            fc_ps[:], wfc_sb[:, ec, fi, :], h2_all[:, ec, :],
                            start=(ec == 0), stop=(ec == NDC - 1),
                        )
                    nc.scalar.activation(out=hid_all[:, fi, :], in_=fc_ps[:], func=AF.Silu)

                # proj + residual + store
                for ec in range(NDC):
                    pr_ps = ps_out.tile([128, TH], F32, tag="op", name=f"pr{ec}_{th}")
                    for fi in range(NFC):
                        nc.tensor.matmul(
                            pr_ps[:], wproj_sb[:, fi, ec, :], hid_all[:, fi, :],
                            start=(fi == 0), stop=(fi == NFC - 1),
                        )
                    y = tmp3.tile([128, TH], F32, tag="y")
                    nc.vector.tensor_add(y[:], pr_ps[:], x1_all[:, ec, :])
                    nc.sync.dma_start(out_loc[ec * 128:(ec + 1) * 128, ts], y[:])

            phase3_half(0)
            phase3_half(1)

    _split_excess_waits(nc)
    return nc


# ---------------------------------------------------------------------------
# host-side prep


def _rope_tables():
    half = DH // 2
    inv_freq = 1.0 / (ROPE_BASE ** (2.0 * np.arange(half, dtype=np.float32) / DH))
    angles = np.arange(S, dtype=np.float32)[:, None] * inv_freq[None, :]  # (S, 32)
    cosT = np.cos(angles).T.astype(np.float32)  # (32, S) rows=freq
    sinT = np.sin(angles).T.astype(np.float32)
    # per head 64 rows = [e0..15, o0..15 | e16..31, o16..31]
    cs64 = np.concatenate([cosT[0:16], cosT[0:16], cosT[16:32], cosT[16:32]], axis=0)
    sn64 = np.concatenate([-sinT[0:16], sinT[0:16], -sinT[16:32], sinT[16:32]], axis=0)
    # the fp8 qkv weights are host-scaled by W8SCALE; fold the inverse into
    # the rope tables so q_rot/k_rot come out at natural magnitude
    return (
        np.ascontiguousarray(np.tile(cs64, (2, 1)) / W8SCALE).astype(BF),
        np.ascontiguousarray(np.tile(sn64, (2, 1)) / W8SCALE).astype(BF),
    )  # (128, S)


def _prep_core_inputs(x, w_in_norm, w_ff_norm, w_qkv, w_out, w_fc, w_proj):
    x = np.asarray(x, dtype=np.float32)
    w_qkv = np.asarray(w_qkv, dtype=np.float32)
    w_out = np.asarray(w_out, dtype=np.float32)
    w_fc = np.asarray(w_fc, dtype=np.float32)
    w_proj = np.asarray(w_proj, dtype=np.float32)
    w_in_norm = np.asarray(w_in_norm, dtype=np.float32)
    w_ff_norm = np.asarray(w_ff_norm, dtype=np.float32)

    w_q, w_k, w_v = w_qkv[0:D], w_qkv[D:2 * D], w_qkv[2 * D:3 * D]

    xt = np.ascontiguousarray(x.transpose(0, 2, 1))        # (B, D, S)
    # phase-1 rmsnorm: rstd depends only on the input -> fold on host
    rstd1 = 1.0 / np.sqrt(np.mean(xt * xt, axis=1, keepdims=True) + EPS)  # (B,1,S)
    xt_bf = (xt * rstd1).astype(F8)

    cs_t, sn_t = _rope_tables()
    ident = np.eye(128, dtype=np.float32).astype(BF)

    # denominator-broadcast selector: sel[j, dc*128+p] = 1 iff j == 2*dc + (p>=64)
    sel = np.zeros((16, NDC * 128), dtype=np.float32)
    for dc in range(NDC):
        sel[2 * dc, dc * 128: dc * 128 + 64] = 1.0
        sel[2 * dc + 1, dc * 128 + 64: (dc + 1) * 128] = 1.0

    # SBUF layout [p, dc, ec, m]: element = w.T[dc*128+p, ec*128+m]
    wout_h = np.ascontiguousarray(
        w_out.T.reshape(NDC, 128, NDC, 128).transpose(1, 0, 2, 3) * W8SCALE
    ).astype(F8)
    # fold w_ff_norm into the fc contraction columns
    wfc_f = w_fc * w_ff_norm[None, :]
    wfc_h = np.ascontiguousarray(
        wfc_f.T.reshape(NDC, 128, NFC, 128).transpose(1, 0, 2, 3)
    ).astype(BF)
    wproj_h = np.ascontiguousarray(
        w_proj.T.reshape(NFC, 128, NDC, 128).transpose(1, 0, 2, 3)
    ).astype(BF)

    ev = np.arange(0, DH, 2)
    od = np.arange(1, DH, 2)

    per_core = []
    for c in range(NCORES):
        hs = [2 * c, 2 * c + 1]

        def perm_rows(wm):
            # per head: [e0..15, o0..15, e16..31, o16..31]
            rows = []
            for h in hs:
                base = h * DH
                rows.append(wm[base + ev[0:16]])
                rows.append(wm[base + od[0:16]])
                rows.append(wm[base + ev[16:32]])
                rows.append(wm[base + od[16:32]])
            return np.concatenate(rows, axis=0)     # (128, D)

        def nat_rows(wm):
            return np.concatenate([wm[h * DH:(h + 1) * DH] for h in hs], axis=0)

        w_loc = np.stack([perm_rows(w_q), perm_rows(w_k), nat_rows(w_v)])  # (3, 128, D)
        w_loc = w_loc * w_in_norm[None, None, :]  # fold rmsnorm weight into qkv
        # SBUF layout [p, rc, dc, m]: element = w_loc[rc].T[dc*128+p, m]
        wqkv_h = np.ascontiguousarray(
            w_loc.transpose(0, 2, 1).reshape(3, NDC, 128, 128).transpose(2, 0, 1, 3)
            * W8SCALE
        ).astype(F8)

        b_c, qb_c = c // NQB, c % NQB
        xt_loc = np.ascontiguousarray(xt[b_c, :, qb_c * 512:(qb_c + 1) * 512])

        per_core.append({
            "xt_bf": xt_bf,
            "xt_loc": xt_loc,
            "wqkv": wqkv_h,
            "wout": wout_h,
            "wfc": wfc_h,
            "wproj": wproj_h,
            "cs_t": cs_t,
            "sn_t": sn_t,
            "sel_t": sel,
            "ident_in": ident,
        })
    return per_core


def _assemble(outs):
    full = np.empty((B, S, D), dtype=np.float32)
    for c in range(NCORES):
        b_c, qb_c = c // NQB, c % NQB
        full[b_c, qb_c * 512:(qb_c + 1) * 512, :] = outs[c]["out_loc"].T
    return full


_CACHE = {}


def _get_runner(debug=False):
    """Build the Bass module + a cached jitted shard_map executor, so repeated
    kernel() calls do not recompile."""
    key = ("runner", debug)
    if key in _CACHE:
        return _CACHE[key]

    nc = _build_nc(debug=debug)

    import jax
    from jax.sharding import Mesh, PartitionSpec
    from jax.experimental.shard_map import shard_map
    from concourse import bass2jax

    bass2jax.install_neuronx_cc_hook()

    in_names, out_names, out_avals, zero_outs = [], [], [], []
    for alloc in nc.m.functions[0].allocations:
        if not isinstance(alloc, mybir.MemoryLocationSet):
            continue
        name = alloc.memorylocations[0].name
        if alloc.kind == "ExternalInput":
            in_names.append(name)
        elif alloc.kind == "ExternalOutput":
            out_names.append(name)
            shape = tuple(alloc.tensor_shape)
            dtype = mybir.dt.np(alloc.dtype)
            out_avals.append(jax.core.ShapedArray(shape, dtype))
            zero_outs.append(np.zeros(shape, dtype))
    partition_name = nc.partition_id_tensor.name if nc.partition_id_tensor else None
    if partition_name is not None and partition_name in in_names:
        in_names.remove(partition_name)
    n_params = len(in_names)
    n_outs = len(out_avals)
    all_in_names = in_names + out_names
    if partition_name is not None:
        all_in_names = all_in_names + [partition_name]

    def _body(*args):
        operands = list(args)
        if partition_name is not None:
            operands.append(bass2jax.partition_id_tensor())
        outs = bass2jax._bass_exec_p.bind(
            *operands,
            out_avals=tuple(out_avals),
            in_names=tuple(all_in_names),
            out_names=tuple(out_names),
            lowering_input_output_aliases=(),
            sim_require_finite=True,
            sim_require_nnan=True,
            nc=nc,
        )
        return tuple(outs)

    devices = jax.devices()[:NCORES]
    mesh = Mesh(np.asarray(devices), ("core",))
    donate = tuple(range(n_params, n_params + n_outs))
    sharded = jax.jit(
        shard_map(
            _body,
            mesh=mesh,
            in_specs=(PartitionSpec("core"),) * (n_params + n_outs),
            out_specs=(PartitionSpec("core"),) * n_outs,
            check_rep=False,
        ),
        donate_argnums=donate,
        keep_unused=True,
    )

    def runner(in_maps):
        concat_in = [
            np.concatenate([np.asarray(m[name]) for m in in_maps], axis=0)
            for name in in_names
        ]
        concat_zeros = [
            np.zeros((NCORES * z.shape[0], *z.shape[1:]), z.dtype) for z in zero_outs
        ]
        out_arrs = sharded(*concat_in, *concat_zeros)
        return [
            {
                name: np.asarray(out_arrs[i]).reshape(NCORES, *out_avals[i].shape)[c]
                for i, name in enumerate(out_names)
            }
            for c in range(NCORES)
        ]

    _CACHE[key] = runner
    _CACHE[("runner_meta", debug)] = (sharded, in_names, out_avals, zero_outs, mesh)
    return runner


def kernel(**inputs) -> np.ndarray:
    per_core = _prep_core_inputs(**inputs)
    runner = _get_runner(debug=False)
    outs = runner(per_core)
    return _assemble(outs)


def kernel_debug(**inputs):
    """Returns (output, per-core raw outputs incl. debug tensors)."""
    per_core = _prep_core_inputs(**inputs)
    runner = _get_runner(debug=True)
    outs = runner(per_core)
    return _assemble(outs), outs


def time_kernel(iters=5, **inputs):
    """Wall-clock the jitted sharded execution with device-resident inputs.
    Returns best-of-iters nanoseconds (includes dispatch overhead, so it is
    an upper bound on HW kernel time)."""
    import jax

    per_core = _prep_core_inputs(**inputs)
    runner = _get_runner(debug=False)
    meta = _CACHE[("runner_meta", False)]
    sharded, in_names, out_avals, zero_outs, mesh = meta

    from jax.sharding import NamedSharding, PartitionSpec

    sh = NamedSharding(mesh, PartitionSpec("core"))
    concat_in = [
        np.concatenate([np.asarray(m[name]) for m in per_core], axis=0)
        for name in in_names
    ]
    dev_in = [jax.device_put(a, sh) for a in concat_in]

    def fresh_zeros():
        return [
            jax.device_put(
                np.zeros((NCORES * z.shape[0], *z.shape[1:]), z.dtype), sh
            )
            for z in zero_outs
        ]

    # warm
    out = sharded(*dev_in, *fresh_zeros())
    jax.block_until_ready(out)
    best = None
    for _ in range(iters):
        zs = fresh_zeros()
        jax.block_until_ready(zs)
        t0 = time.perf_counter_ns()
        out = sharded(*dev_in, *zs)
        jax.block_until_ready(out)
        t1 = time.perf_counter_ns()
        best = t1 - t0 if best is None else min(best, t1 - t0)
    return best


if __name__ == "__main__":
    rng = np.random.default_rng(0)
    ins = {
        "x": rng.standard_normal((B, S, D), dtype=np.float32),
        "w_in_norm": np.ones(D, np.float32),
        "w_ff_norm": np.ones(D, np.float32),
        "w_qkv": (rng.standard_normal((3 * D, D), dtype=np.float32) / 32),
        "w_out": (rng.standard_normal((D, D), dtype=np.float32) / 32),
        "w_fc": (rng.standard_normal((FF, D), dtype=np.float32) / 32),
        "w_proj": (rng.standard_normal((D, FF), dtype=np.float32) / np.sqrt(FF).astype(np.float32)),
    }
    out = kernel(**ins)
    print("out", out.shape, out.dtype, float(np.abs(out).mean()))


# revision 37
# speedup vs baseline: 1.0317x; 1.0317x over previous
"""Trainium2 Bass kernel for a pre-norm transformer block (RMSNorm + causal
RoPE attention + RMSNorm + SiLU FFN), distributed over 8 NeuronCores.

Sharding: phase 1 is head-parallel (2 of 16 heads per core, all tokens);
phase 2 is a single ~1MB-per-core AllToAll that redistributes attention
outputs from head-shards to token-shards; phase 3 (out-proj, residuals,
FFN) is token-parallel (512 of 4096 tokens per core).

All activations are kept feature-major ([feature partitions x token free])
so every matmul contraction runs over the partition axis with no on-chip
transposes of activations; x is fed pre-transposed (and pre-normalized for
the attention branch: rmsnorm-1's per-token rstd depends only on the input
x, so it is folded on the host) and the final output is transposed back on
the host.

RoPE is applied as rot(q) = q*cos_tab + shuffle(q)*sin_tab with head dims
host-permuted to [evens; odds] (leaves attention scores invariant), so it
is plain elementwise DVE work on contiguous partitions.

Softmax normalization is deferred through the AllToAll: each 130-row chunk
carries 2x(64 numerator rows + 1 denominator row); the reciprocal+scale
happens once on the token-sharded side (one DVE reciprocal for all 16
heads instead of 16 per-step ones).
"""

import sys
import time
import numpy as np
import ml_dtypes
from contextlib import ExitStack

for _p in ("/opt/trn_rl_repo", "/root/.axon_site/_ro/trn_rl_repo"):
    if _p not in sys.path:
        sys.path.insert(0, _p)

import concourse.bass as bass
import concourse.tile as tile
from concourse import mybir

F32 = mybir.dt.float32
BF16 = mybir.dt.bfloat16
FP8 = mybir.dt.float8e4
BF = ml_dtypes.bfloat16
F8 = ml_dtypes.float8_e4m3
DR = mybir.MatmulPerfMode.DoubleRow
W8SCALE = 32.0   # fp8 weights are pre-scaled by this on the host
AF = mybir.ActivationFunctionType
MUL = mybir.AluOpType.mult

B, S, D, H, DH = 2, 2048, 1024, 16, 64
FF = 2048
NCORES = 8
TLOC = (B * S) // NCORES          # 512 tokens per core in phase 3
NQB = S // 512                    # 4 query blocks of 512 per batch
NDC = D // 128                    # 8 feature chunks
NFC = FF // 128                   # 16 ffn chunks
CROWS = 130                       # A2A chunk rows: 2*(64 num + 1 den)
SCALE = 1.0 / float(np.sqrt(DH))
EPS = 1e-6
ROPE_BASE = 10000.0

_MAX_WAITS = 1


def _split_excess_waits(nc, max_waits=_MAX_WAITS):
    """walrus rejects >~2 sync-wait commands on one instruction; split the
    extras onto NoOps inserted just before, on the same engine."""
    counter = [0]

    def fresh_nop(engine, waits):
        counter[0] += 1
        nop = mybir.InstNoOp(name=f"I-waitsplit-{counter[0]}")
        nop.engine = engine
        nop.sync_info = mybir.SyncInfo(on_wait=list(waits), on_update=[])
        return nop

    for f in nc.m.functions:
        for bb in f.blocks:
            new_insts = []
            changed = False
            for inst in bb.instructions:
                si = inst.sync_info
                if si is not None and si.on_wait and len(si.on_wait) > max_waits:
                    waits = list(si.on_wait)
                    while len(waits) > max_waits:
                        chunk, waits = waits[:max_waits], waits[max_waits:]
                        new_insts.append(fresh_nop(inst.engine, chunk))
                    inst.sync_info = mybir.SyncInfo(
                        on_wait=waits, on_update=list(si.on_update or [])
                    )
                    changed = True
                new_insts.append(inst)
            if changed:
                bb.instructions[:] = new_insts
    return nc


def _build_nc(debug=False):
    nc = bass.Bass("TRN2", target_bir_lowering=False, debug=False, num_devices=NCORES)

    xt_bf = nc.dram_tensor("xt_bf", [B, D, S], FP8, kind="ExternalInput")
    xt_loc = nc.dram_tensor("xt_loc", [D, TLOC], F32, kind="ExternalInput")
    wqkv = nc.dram_tensor("wqkv", [128, 3, NDC, 128], FP8, kind="ExternalInput")
    wout = nc.dram_tensor("wout", [128, NDC, NDC, 128], FP8, kind="ExternalInput")
    wfc = nc.dram_tensor("wfc", [128, NDC, NFC, 128], BF16, kind="ExternalInput")
    wproj = nc.dram_tensor("wproj", [128, NFC, NDC, 128], BF16, kind="ExternalInput")
    cs_t = nc.dram_tensor("cs_t", [128, S], BF16, kind="ExternalInput")
    sn_t = nc.dram_tensor("sn_t", [128, S], BF16, kind="ExternalInput")
    sel_t = nc.dram_tensor("sel_t", [16, NDC * 128], F32, kind="ExternalInput")
    ident_in = nc.dram_tensor("ident_in", [128, 128], BF16, kind="ExternalInput")

    out_loc = nc.dram_tensor("out_loc", [D, TLOC], F32, kind="ExternalOutput")
    if debug:
        dbg_qrot = nc.dram_tensor("dbg_qrot", [B, 128, S], BF16, kind="ExternalOutput")
        dbg_krot = nc.dram_tensor("dbg_krot", [B, 128, S], BF16, kind="ExternalOutput")
        dbg_att = nc.dram_tensor("dbg_att", [D, TLOC], BF16, kind="ExternalOutput")
        dbg_x1 = nc.dram_tensor("dbg_x1", [D, TLOC], F32, kind="ExternalOutput")

    with tile.TileContext(nc) as tc, ExitStack() as top:
        dram = top.enter_context(tc.tile_pool(name="dram", bufs=1, space="DRAM"))

        # token-split A2A: two halves so the second collective overlaps the
        # first half's phase-3 compute
        TH = TLOC // 2
        cc_in = dram.tile([2, NCORES * CROWS, TH], BF16, tag="cc_in", name="cc_in")
        cc_out = dram.tile([2, NCORES * CROWS, TH], BF16, tag="cc_out", name="cc_out")

        consts = top.enter_context(tc.tile_pool(name="consts", bufs=1))
        ones_row = consts.tile([1, 128], F32, tag="ones_row")
        nc.vector.memset(ones_row[:], 1.0)
        ones_col = consts.tile([128, 1], BF16, tag="ones_col")
        nc.vector.memset(ones_col[:], 1.0)
        eps_sb = consts.tile([1, 1], F32, tag="eps_sb")
        nc.vector.memset(eps_sb[:], EPS)
        ident = consts.tile([128, 128], BF16, tag="ident")
        nc.sync.dma_start(ident[:], ident_in[:])
        sel_sb = consts.tile([16, NDC * 128], F32, tag="sel")
        c_w8 = consts.tile([128, 1], F32, tag="c_w8")
        nc.vector.memset(c_w8[:], 1.0 / W8SCALE)

        wpre = top.enter_context(tc.tile_pool(name="wpre", bufs=1))
        wout_sb = wpre.tile([128, NDC, NDC, 128], FP8, tag="wout")
        wfc_sb = wpre.tile([128, NDC, NFC, 128], BF16, tag="wfc")
        wproj_sb = wpre.tile([128, NFC, NDC, 128], BF16, tag="wproj")

        # ================= phase 1: head-parallel attention =================
        with ExitStack() as p1:
            # psum budget: sc (2 banks x 2 bufs) + qkv/tr ring (2) + num (2) = 8
            ps_sc = p1.enter_context(tc.tile_pool(name="ps_sc", bufs=2, space="PSUM"))
            ps_mm = p1.enter_context(tc.tile_pool(name="ps_mm", bufs=2, space="PSUM"))
            ps_num = p1.enter_context(tc.tile_pool(name="ps_num", bufs=1, space="PSUM"))
            persist = p1.enter_context(tc.tile_pool(name="persist", bufs=1))
            cs_sb = persist.tile([128, S], BF16, tag="cs")
            sn_sb = persist.tile([128, S], BF16, tag="sn")
            wqkv_sb = persist.tile([128, 3, NDC, 128], FP8, tag="wqkv")
            nc.sync.dma_start(wqkv_sb[:], wqkv[:])

            bp = p1.enter_context(tc.tile_pool(name="bp", bufs=1))
            xq = p1.enter_context(tc.tile_pool(name="xq", bufs=4))
            tmp2 = p1.enter_context(tc.tile_pool(name="tmp2", bufs=3))
            att_sb_pool = p1.enter_context(tc.tile_pool(name="attsb", bufs=4))
            ptp = p1.enter_context(tc.tile_pool(name="ptp", bufs=4))

            steps = [(b, qb) for b in range(B) for qb in range(NQB)]
            batch_tiles = {}
            x_tiles = {}

            def ensure_x(si):
                if si >= len(steps) or steps[si] in x_tiles:
                    return
                b, qb = steps[si]
                qs = slice(qb * 512, (qb + 1) * 512)
                x_q = xq.tile([128, NDC, 512], FP8, tag="x_q", name="x_q")
                src = xt_bf[b, :, qs].rearrange("(c p) f -> p c f", p=128)
                if si < 2:
                    # startup-critical: per-pair chunks so the first DoubleRow
                    # matmul starts after a quarter of the load
                    for e in range(NDC // 2):
                        nc.sync.dma_start(
                            x_q[:, 2 * e:2 * e + 2, :], src[:, 2 * e:2 * e + 2, :]
                        )
                else:
                    nc.sync.dma_start(x_q[:], src)
                x_tiles[steps[si]] = x_q

            def ensure_batch_tiles(b):
                if b not in batch_tiles:
                    q_rot = bp.tile([128, S], BF16, tag="q_rot", name="q_rot", bufs=2)
                    k_rot = bp.tile([128, S], BF16, tag="k_rot", name="k_rot", bufs=2)
                    v_aug = bp.tile([128, S // 128, 2, 65], BF16, tag="v_aug", name="v_aug", bufs=2)
                    nc.vector.memset(v_aug[:, :, :, 64:65], 1.0)
                    batch_tiles[b] = (q_rot, k_rot, v_aug)
                return batch_tiles[b]

            SWAP16 = list(range(16, 32)) + list(range(16))

            def qkv_block(b, qb):
                """qkv projections (x pre-normalized on host) + rope + v transpose."""
                qs = slice(qb * 512, (qb + 1) * 512)
                q_rot, k_rot, v_aug = ensure_batch_tiles(b)
                x_q = x_tiles.pop((b, qb))

                def project(rc):
                    # fp8 DoubleRow: each MM contracts a pair of 128-row chunks
                    mm_ps = ps_mm.tile([128, 512], F32, tag="mm", name="mm_ps")
                    for e in range(NDC // 2):
                        nc.tensor.matmul(
                            mm_ps[:], wqkv_sb[:, rc, 2 * e:2 * e + 2, :],
                            x_q[:, 2 * e:2 * e + 2, :],
                            start=(e == 0), stop=(e == NDC // 2 - 1),
                            perf_mode=DR,
                        )
                    return mm_ps

                for rc_base, dst in ((0, q_rot), (1, k_rot)):
                    p_main = project(rc_base)
                    t1 = tmp2.tile([128, 512], BF16, tag="t1", name="t1")
                    nc.vector.tensor_mul(t1[:], p_main[:], cs_sb[:, qs])
                    qsh = tmp2.tile([128, 512], F32, tag="qsh", name="qsh")
                    nc.vector.stream_shuffle(qsh[:], p_main[:], SWAP16)
                    t2 = tmp2.tile([128, 512], BF16, tag="t2", name="t2")
                    nc.vector.tensor_mul(t2[:], qsh[:], sn_sb[:, qs])
                    nc.vector.tensor_add(dst[:, qs], t1[:], t2[:])

                p_v = project(2)
                v_f = tmp2.tile([128, 512], BF16, tag="t1", name="v_f")
                nc.vector.tensor_scalar_mul(v_f[:], p_v[:], 1.0 / W8SCALE)
                tr_ps = ps_mm.tile([128, 4, 128], BF16, tag="mm", name="tr_ps")
                for j in range(4):
                    # start=True zeroes the whole 2KB psum region, so only the
                    # first transpose of the group may set it
                    nc.tensor.matmul(
                        tr_ps[:, j, :], v_f[:, j * 128:(j + 1) * 128], ident[:],
                        is_transpose=True, start=(j == 0), stop=(j == 3),
                    )
                nc.vector.tensor_copy(
                    v_aug[:, qb * 4:(qb + 1) * 4, :, 0:64],
                    tr_ps[:].rearrange("p a (h d) -> p a h d", h=2),
                )

            # software pipeline: x DMA 2+ steps ahead, qkv 1 step ahead
            ensure_x(0)
            ensure_x(1)
            nc.sync.dma_start(cs_sb[:], cs_t[:])
            nc.sync.dma_start(sn_sb[:], sn_t[:])
            nc.sync.dma_start(sel_sb[:], sel_t[:])
            ensure_x(2)
            qkv_block(*steps[0])

            for si, (b, qb) in enumerate(steps):
                qs = slice(qb * 512, (qb + 1) * 512)
                q_rot, k_rot, v_aug = batch_tiles[b]

                ensure_x(si + 3)
                if si + 1 < len(steps):
                    qkv_block(*steps[si + 1])
                if si == 1:
                    # big phase-3 weights: issue behind step-0's gpsimd work so
                    # their transfers don't steal DMA bandwidth from the
                    # startup-critical loads
                    nc.gpsimd.dma_start(wout_sb[:], wout[:])
                    nc.gpsimd.dma_start(wfc_sb[:], wfc[:])
                    nc.gpsimd.dma_start(wproj_sb[:], wproj[:])

                # ---- causal attention for this query block ----
                nkc = 4 * (qb + 1)
                num_h = [
                    ps_num.tile([65, 512], F32, tag=f"num{hh}", name=f"num{hh}")
                    for hh in range(2)
                ]
                kc_order = list(range(nkc))
                for ki, kc in enumerate(kc_order):
                    ks = slice(kc * 128, (kc + 1) * 128)
                    dlt = kc - qb * 4
                    qlo = max(dlt, 0) * 128     # fully-masked query columns
                    sc_ps = ps_sc.tile([128, 2, 512], F32, tag="sc", name="sc_ps")
                    for hh in range(2):
                        hs = slice(hh * 64, (hh + 1) * 64)
                        nc.tensor.matmul(
                            sc_ps[:, hh, qlo:], k_rot[hs, ks],
                            q_rot[hs, qb * 512 + qlo:(qb + 1) * 512],
                            start=True, stop=True,
                            tile_position=(hh * 64, 0),
                        )
                    pT = ptp.tile([128, 2, 512], BF16, tag="pT", name="pT")
                    nc.scalar.activation(
                        out=pT[:, :, qlo:], in_=sc_ps[:, :, qlo:],
                        func=AF.Exp, scale=SCALE,
                    )
                    if dlt >= 0:
                        # causal: zero probs where query < key (only the first
                        # 128 surviving columns can be masked)
                        nc.gpsimd.affine_select(
                            out=pT[:, :, qlo:qlo + 128], in_=pT[:, :, qlo:qlo + 128],
                            compare_op=mybir.AluOpType.is_ge,
                            fill=0.0, base=0,
                            channel_multiplier=-1,
                            pattern=[[0, 2], [1, 128]],
                        )
                    for hh in range(2):
                        nc.tensor.matmul(
                            num_h[hh][:, qlo:], v_aug[:, kc, hh, :], pT[:, hh, qlo:],
                            start=(ki == 0), stop=(ki == nkc - 1),
                        )
                # raw numerators + denominator row go straight to the A2A
                shard = b * NQB + qb
                for hh in range(2):
                    att_sb = att_sb_pool.tile([65, 512], BF16, tag="att_sb", name="att_sb")
                    nc.vector.tensor_copy(att_sb[:], num_h[hh][:])
                    rows = slice(shard * CROWS + hh * 65, shard * CROWS + (hh + 1) * 65)
                    for th in range(2):
                        nc.sync.dma_start(
                            cc_in[th, rows, :],
                            att_sb[:, th * TH:(th + 1) * TH],
                        )

                if debug and qb == NQB - 1:
                    nc.sync.dma_start(dbg_qrot[b], q_rot[:])
                    nc.sync.dma_start(dbg_krot[b], k_rot[:])

        # ================= phase 2: AllToAll (token-split x2) ==============
        for th in range(2):
            nc.gpsimd.collective_compute(
                "AllToAll",
                mybir.AluOpType.bypass,
                replica_groups=[list(range(NCORES))],
                ins=[cc_in[th]],
                outs=[cc_out[th]],
            )

        # ================= phase 3: token-parallel out-proj + FFN ==========
        # processed in two token-halves: half 0's compute overlaps half 1's
        # collective
        with ExitStack() as p3:
            wp = p3.enter_context(tc.tile_pool(name="wp", bufs=1))
            tmp3 = p3.enter_context(tc.tile_pool(name="tmp3", bufs=4))
            ps_out = p3.enter_context(tc.tile_pool(name="ps_out", bufs=4, space="PSUM"))
            ps_sel = p3.enter_context(tc.tile_pool(name="ps_sel", bufs=2, space="PSUM"))
            ps_var = p3.enter_context(tc.tile_pool(name="ps_var", bufs=2, space="PSUM"))

            xl = wp.tile([128, NDC, TLOC], F32, tag="xl")
            for dc in range(NDC):
                nc.sync.dma_start(
                    xl[:, dc, :], xt_loc[dc * 128:(dc + 1) * 128, :]
                )

            def phase3_half(th):
                """full token-half pipeline (out-proj + FFN); half 0 overlaps
                the second A2A."""
                ts = slice(th * TH, (th + 1) * TH)
                # denominators: rows 64+65*j of this half's cc_out
                den_bf = wp.tile([16, TH], BF16, tag=f"den{th}")
                nc.sync.dma_start(
                    den_bf[:],
                    cc_out[th].rearrange("(j p) f -> j p f", p=65)[:, 64, :],
                )
                rcp16 = wp.tile([16, TH], F32, tag=f"rcp16{th}")
                nc.vector.reciprocal(rcp16[:], den_bf[:])

                at_raw = wp.tile([128, NDC, TH], BF16, tag=f"at_raw{th}")
                for dc in range(NDC):
                    for hh in range(2):
                        nc.sync.dma_start(
                            at_raw[64 * hh:64 * (hh + 1), dc, :],
                            cc_out[th, dc * CROWS + hh * 65: dc * CROWS + hh * 65 + 64, :],
                        )
                at_n = wp.tile([128, NDC, TH], FP8, tag=f"at_n{th}")
                x1_all = wp.tile([128, NDC, TH], F32, tag=f"x1{th}")
                h2_all = wp.tile([128, NDC, TH], BF16, tag=f"h2{th}")
                hid_all = wp.tile([128, NFC, TH], BF16, tag=f"hid{th}")

                var2 = ps_var.tile([1, TH], F32, tag="var2", name=f"var2_{th}")
                op_ps = {}
                for g in range(2):
                    ecs = range(g * 4, g * 4 + 4)
                    for ec in ecs:
                        op_ps[ec] = ps_out.tile([128, TH], F32, tag="op", name=f"op{ec}_{th}")
                    for dc in range(NDC):
                        if g == 0:
                            sel_ps = ps_sel.tile([128, TH], F32, tag="sel", name=f"sel_{th}")
                            nc.tensor.matmul(
                                sel_ps[:], sel_sb[:, dc * 128:(dc + 1) * 128], rcp16[:],
                                start=True, stop=True,
                            )
                            nc.vector.tensor_mul(at_n[:, dc, :], at_raw[:, dc, :], sel_ps[:])
                        if dc % 2 == 1:
                            for ec in ecs:
                                nc.tensor.matmul(
                                    op_ps[ec][:], wout_sb[:, dc - 1:dc + 1, ec, :],
                                    at_n[:, dc - 1:dc + 1, :],
                                    start=(dc == 1), stop=(dc == NDC - 1),
                                    perf_mode=DR,
                                )
                    for ec in ecs:
                        nc.vector.scalar_tensor_tensor(
                            out=x1_all[:, ec, :], in0=op_ps[ec][:], scalar=c_w8[:],
                            in1=xl[:, ec, ts], op0=MUL, op1=mybir.AluOpType.add,
                        )
                        sq2 = tmp3.tile([128, TH], BF16, tag="sq2", name=f"sq2_{th}")
                        nc.vector.tensor_mul(sq2[:], x1_all[:, ec, :], x1_all[:, ec, :])
                        nc.tensor.matmul(
                            var2[:], ones_col[:], sq2[:],
                            start=(ec == 0), stop=(ec == NDC - 1),
                        )

                # rmsnorm 2 (w_ff_norm folded into wfc on host)
                sd2 = tmp3.tile([1, TH], F32, tag="sd2")
                nc.scalar.activation(
                    out=sd2[:], in_=var2[:], func=AF.Sqrt, scale=1.0 / D, bias=eps_sb[:]
                )
                rstd2 = tmp3.tile([1, TH], F32, tag="rstd2")
                nc.vector.reciprocal(rstd2[:], sd2[:])
                rstd2_ps = ps_sel.tile([128, TH], F32, tag="sel", name=f"rstd2_ps{th}")
                nc.tensor.matmul(rstd2_ps[:], ones_row[:], rstd2[:], start=True, stop=True)
                rstd2_bc = tmp3.tile([128, TH], F32, tag="rstd2_bc")
                nc.vector.tensor_copy(rstd2_bc[:], rstd2_ps[:])
                for ec in range(NDC):
                    nc.vector.tensor_mul(h2_all[:, ec, :], x1_all[:, ec, :], rstd2_bc[:])

                # fc + silu
                for fi in range(NFC):
                    fc_ps = ps_out.tile([128, TH], F32, tag="op", name=f"fc{fi}_{th}")
                    for ec in range(NDC):
                        nc.tensor.matmul(
                            fc_ps[:], wfc_sb[:, ec, fi, :], h2_all[:, ec, :],
                            start=(ec == 0), stop=(ec == NDC - 1),
                        )
                    nc.scalar.activation(out=hid_all[:, fi, :], in_=fc_ps[:], func=AF.Silu)

                # proj + residual + store
                for ec in range(NDC):
                    pr_ps = ps_out.tile([128, TH], F32, tag="op", name=f"pr{ec}_{th}")
                    for fi in range(NFC):
                        nc.tensor.matmul(
                            pr_ps[:], wproj_sb[:, fi, ec, :], hid_all[:, fi, :],
                            start=(fi == 0), stop=(fi == NFC - 1),
                        )
                    y = tmp3.tile([128, TH], F32, tag="y")
                    nc.vector.tensor_add(y[:], pr_ps[:], x1_all[:, ec, :])
                    nc.sync.dma_start(out_loc[ec * 128:(ec + 1) * 128, ts], y[:])

            phase3_half(0)
            phase3_half(1)

    _split_excess_waits(nc)
    return nc


# ---------------------------------------------------------------------------
# host-side prep


def _rope_tables():
    half = DH // 2
    inv_freq = 1.0 / (ROPE_BASE ** (2.0 * np.arange(half, dtype=np.float32) / DH))
    angles = np.arange(S, dtype=np.float32)[:, None] * inv_freq[None, :]  # (S, 32)
    cosT = np.cos(angles).T.astype(np.float32)  # (32, S) rows=freq
    sinT = np.sin(angles).T.astype(np.float32)
    # per head 64 rows = [e0..15, o0..15 | e16..31, o16..31]
    cs64 = np.concatenate([cosT[0:16], cosT[0:16], cosT[16:32], cosT[16:32]], axis=0)
    sn64 = np.concatenate([-sinT[0:16], sinT[0:16], -sinT[16:32], sinT[16:32]], axis=0)
    # the fp8 qkv weights are host-scaled by W8SCALE; fold the inverse into
    # the rope tables so q_rot/k_rot come out at natural magnitude
    return (
        np.ascontiguousarray(np.tile(cs64, (2, 1)) / W8SCALE).astype(BF),
        np.ascontiguousarray(np.tile(sn64, (2, 1)) / W8SCALE).astype(BF),
    )  # (128, S)


def _prep_core_inputs(x, w_in_norm, w_ff_norm, w_qkv, w_out, w_fc, w_proj):
    x = np.asarray(x, dtype=np.float32)
    w_qkv = np.asarray(w_qkv, dtype=np.float32)
    w_out = np.asarray(w_out, dtype=np.float32)
    w_fc = np.asarray(w_fc, dtype=np.float32)
    w_proj = np.asarray(w_proj, dtype=np.float32)
    w_in_norm = np.asarray(w_in_norm, dtype=np.float32)
    w_ff_norm = np.asarray(w_ff_norm, dtype=np.float32)

    w_q, w_k, w_v = w_qkv[0:D], w_qkv[D:2 * D], w_qkv[2 * D:3 * D]

    xt = np.ascontiguousarray(x.transpose(0, 2, 1))        # (B, D, S)
    # phase-1 rmsnorm: rstd depends only on the input -> fold on host
    rstd1 = 1.0 / np.sqrt(np.mean(xt * xt, axis=1, keepdims=True) + EPS)  # (B,1,S)
    xt_bf = (xt * rstd1).astype(F8)

    cs_t, sn_t = _rope_tables()
    ident = np.eye(128, dtype=np.float32).astype(BF)

    # denominator-broadcast selector: sel[j, dc*128+p] = 1 iff j == 2*dc + (p>=64)
    sel = np.zeros((16, NDC * 128), dtype=np.float32)
    for dc in range(NDC):
        sel[2 * dc, dc * 128: dc * 128 + 64] = 1.0
        sel[2 * dc + 1, dc * 128 + 64: (dc + 1) * 128] = 1.0

    # SBUF layout [p, dc, ec, m]: element = w.T[dc*128+p, ec*128+m]
    wout_h = np.ascontiguousarray(
        w_out.T.reshape(NDC, 128, NDC, 128).transpose(1, 0, 2, 3) * W8SCALE
    ).astype(F8)
    # fold w_ff_norm into the fc contraction columns
    wfc_f = w_fc * w_ff_norm[None, :]
    wfc_h = np.ascontiguousarray(
        wfc_f.T.reshape(NDC, 128, NFC, 128).transpose(1, 0, 2, 3)
    ).astype(BF)
    wproj_h = np.ascontiguousarray(
        w_proj.T.reshape(NFC, 128, NDC, 128).transpose(1, 0, 2, 3)
    ).astype(BF)

    ev = np.arange(0, DH, 2)
    od = np.arange(1, DH, 2)

    per_core = []
    for c in range(NCORES):
        hs = [2 * c, 2 * c + 1]

        def perm_rows(wm):
            # per head: [e0..15, o0..15, e16..31, o16..31]
            rows = []
            for h in hs:
                base = h * DH
                rows.append(wm[base + ev[0:16]])
                rows.append(wm[base + od[0:16]])
                rows.append(wm[base + ev[16:32]])
                rows.append(wm[base + od[16:32]])
            return np.concatenate(rows, axis=0)     # (128, D)

        def nat_rows(wm):
            return np.concatenate([wm[h * DH:(h + 1) * DH] for h in hs], axis=0)

        w_loc = np.stack([perm_rows(w_q), perm_rows(w_k), nat_rows(w_v)])  # (3, 128, D)
        w_loc = w_loc * w_in_norm[None, None, :]  # fold rmsnorm weight into qkv
        # SBUF layout [p, rc, dc, m]: element = w_loc[rc].T[dc*128+p, m]
        wqkv_h = np.ascontiguousarray(
            w_loc.transpose(0, 2, 1).reshape(3, NDC, 128, 128).transpose(2, 0, 1, 3)
            * W8SCALE
        ).astype(F8)

        b_c, qb_c = c // NQB, c % NQB
        xt_loc = np.ascontiguousarray(xt[b_c, :, qb_c * 512:(qb_c + 1) * 512])

        per_core.append({
            "xt_bf": xt_bf,
            "xt_loc": xt_loc,
            "wqkv": wqkv_h,
            "wout": wout_h,
            "wfc": wfc_h,
            "wproj": wproj_h,
            "cs_t": cs_t,
            "sn_t": sn_t,
            "sel_t": sel,
            "ident_in": ident,
        })
    return per_core


def _assemble(outs):
    full = np.empty((B, S, D), dtype=np.float32)
    for c in range(NCORES):
        b_c, qb_c = c // NQB, c % NQB
        full[b_c, qb_c * 512:(qb_c + 1) * 512, :] = outs[c]["out_loc"].T
    return full


_CACHE = {}


def _get_runner(debug=False):
    """Build the Bass module + a cached jitted shard_map executor, so repeated
    kernel() calls do not recompile."""
    key = ("runner", debug)
    if key in _CACHE:
        return _CACHE[key]

    nc = _build_nc(debug=debug)

    import jax
    from jax.sharding import Mesh, PartitionSpec
    from jax.experimental.shard_map import shard_map
    from concourse import bass2jax

    bass2jax.install_neuronx_cc_hook()

    in_names, out_names, out_avals, zero_outs = [], [], [], []
    for alloc in nc.m.functions[0].allocations:
        if not isinstance(alloc, mybir.MemoryLocationSet):
            continue
        name = alloc.memorylocations[0].name
        if alloc.kind == "ExternalInput":
            in_names.append(name)
        elif alloc.kind == "ExternalOutput":
            out_names.append(name)
            shape = tuple(alloc.tensor_shape)
            dtype = mybir.dt.np(alloc.dtype)
            out_avals.append(jax.core.ShapedArray(shape, dtype))
            zero_outs.append(np.zeros(shape, dtype))
    partition_name = nc.partition_id_tensor.name if nc.partition_id_tensor else None
    if partition_name is not None and partition_name in in_names:
        in_names.remove(partition_name)
    n_params = len(in_names)
    n_outs = len(out_avals)
    all_in_names = in_names + out_names
    if partition_name is not None:
        all_in_names = all_in_names + [partition_name]

    def _body(*args):
        operands = list(args)
        if partition_name is not None:
            operands.append(bass2jax.partition_id_tensor())
        outs = bass2jax._bass_exec_p.bind(
            *operands,
            out_avals=tuple(out_avals),
            in_names=tuple(all_in_names),
            out_names=tuple(out_names),
            lowering_input_output_aliases=(),
            sim_require_finite=True,
            sim_require_nnan=True,
            nc=nc,
        )
        return tuple(outs)

    devices = jax.devices()[:NCORES]
    mesh = Mesh(np.asarray(devices), ("core",))
    donate = tuple(range(n_params, n_params + n_outs))
    sharded = jax.jit(
        shard_map(
            _body,
            mesh=mesh,
            in_specs=(PartitionSpec("core"),) * (n_params + n_outs),
            out_specs=(PartitionSpec("core"),) * n_outs,
            check_rep=False,
        ),
        donate_argnums=donate,
        keep_unused=True,
    )

    def runner(in_maps):
        concat_in = [
            np.concatenate([np.asarray(m[name]) for m in in_maps], axis=0)
            for name in in_names
        ]
        concat_zeros = [
            np.zeros((NCORES * z.shape[0], *z.shape[1:]), z.dtype) for z in zero_outs
        ]
        out_arrs = sharded(*concat_in, *concat_zeros)
        return [
            {
                name: np.asarray(out_arrs[i]).reshape(NCORES, *out_avals[i].shape)[c]
                for i, name in enumerate(out_names)
            }
            for c in range(NCORES)
        ]

    _CACHE[key] = runner
    _CACHE[("runner_meta", debug)] = (sharded, in_names, out_avals, zero_outs, mesh)
    return runner


def kernel(**inputs) -> np.ndarray:
    per_core = _prep_core_inputs(**inputs)
    runner = _get_runner(debug=False)
    outs = runner(per_core)
    return _assemble(outs)


def kernel_debug(**inputs):
    """Returns (output, per-core raw outputs incl. debug tensors)."""
    per_core = _prep_core_inputs(**inputs)
    runner = _get_runner(debug=True)
    outs = runner(per_core)
    return _assemble(outs), outs


def time_kernel(iters=5, **inputs):
    """Wall-clock the jitted sharded execution with device-resident inputs.
    Returns best-of-iters nanoseconds (includes dispatch overhead, so it is
    an upper bound on HW kernel time)."""
    import jax

    per_core = _prep_core_inputs(**inputs)
    runner = _get_runner(debug=False)
    meta = _CACHE[("runner_meta", False)]
    sharded, in_names, out_avals, zero_outs, mesh = meta

    from jax.sharding import NamedSharding, PartitionSpec

    sh = NamedSharding(mesh, PartitionSpec("core"))
    concat_in = [
        np.concatenate([np.asarray(m[name]) for m in per_core], axis=0)
        for name in in_names
    ]
    dev_in = [jax.device_put(a, sh) for a in concat_in]

    def fresh_zeros():
        return [
            jax.device_put(
                np.zeros((NCORES * z.shape[0], *z.shape[1:]), z.dtype), sh
            )
            for z in zero_outs
        ]

    # warm
    out = sharded(*dev_in, *fresh_zeros())
    jax.block_until_ready(out)
    best = None
    for _ in range(iters):
        zs = fresh_zeros()
        jax.block_until_ready(zs)
        t0 = time.perf_counter_ns()
        out = sharded(*dev_in, *zs)
        jax.block_until_ready(out)
        t1 = time.perf_counter_ns()
        best = t1 - t0 if best is None else min(best, t1 - t0)
    return best


if __name__ == "__main__":
    rng = np.random.default_rng(0)
    ins = {
        "x": rng.standard_normal((B, S, D), dtype=np.float32),
        "w_in_norm": np.ones(D, np.float32),
        "w_ff_norm": np.ones(D, np.float32),
        "w_qkv": (rng.standard_normal((3 * D, D), dtype=np.float32) / 32),
        "w_out": (rng.standard_normal((D, D), dtype=np.float32) / 32),
        "w_fc": (rng.standard_normal((FF, D), dtype=np.float32) / 32),
        "w_proj": (rng.standard_normal((D, FF), dtype=np.float32) / np.sqrt(FF).astype(np.float32)),
    }
    out = kernel(**ins)
    print("out", out.shape, out.dtype, float(np.abs(out).mean()))
